# revision 1
# baseline (speedup 1.0000x reference)
"""Trainium2 Bass kernel for nn_DifferentialAttention (sparse attention).

Reference computation (per batch element b):
    Q = x @ Wq + bq ; K = x @ Wk + bk ; V = x @ Wv + bv        [S, KD]
    scores  = Q @ K^T                                          [S, S]
    weights = softmax(scores, axis=-1)
    mask    = weights > mean(weights, axis=-1, keepdims=True)
    out     = (weights * mask) @ V                             [S, KD]

Single-den-pass formulation ([j,i] layout throughout):
  * mean(softmax row) == 1/S exactly, so mask_ij = [w_ij > 1/S]
      = [s_ij > t_i],  t_i = ln(den_i) + M - ln(S),
    with  e_ij = exp(s_ij - M), den_i = sum_j e_ij.
  * out_i = (sum_j e_ij * mask_ij * V_j) / den_i   -- M cancels.
  Per row-block h (1024 query rows), per j-chunk (128 keys):
    pass A:  u = K16^T Q16      (fp16 matmul -> PSUM f32)
             e = exp(u - M)     -> bf16 SBUF tile (kept for pass B)
             den += ones^T e    (PE column-sum accumulation)
    t_row  = -(ln(den) + M - lnS) as fp16 -> aug row KD of Q16 (a row
             vector, so no transpose); aug row KD of K16 holds ones.
    pass B:  u' = K16aug^T Q16aug = s - t_i   (PE, recomputed scores)
             g  = (u' > 0) * e   (ONE DVE stt: psum,sbuf -> bf16)
             O += V16^T g        (PE accumulation)
    epilogue: transpose O to [i, KD], scale rows by 1/den, DMA out.

  fp16 scores give |ds| ~ 4e-3 => ~0.4% weight error; total rel err ~6e-3
  (budget 2e-2).  Masking costs ONE DVE op per tile: measured stt =
  1.13us/[128,1024]; tt/stt get no DVE 2x mode for bf16, GpSimd is 1.9us,
  and concurrent DVE+GpSimd degrade ~1.5-2.6x (SBUF port contention).

Scheduling: the PE p-state ramps to 2.4 GHz only under continuous issue
(~3us); any dependency stall drops it to 1.2 GHz.  So phases are paired so
the PE always has ~1.7us of independent work per j-chunk:
    P0: projections (PE-dense),   P1: A(h0) || B(h1 of PREVIOUS repeat),
    P2: A(h1) || B(h0),           per-h epilogues after P1/P2.
Accumulation matmuls (den, O) are emitted LAG=2 chunks behind their
producers so they never wait on Act/DVE latency.  Inputs arrive fp16 from
the host (halves DMA and removes all big casts).

Sharding: 8 cores = (batch b in 0..3) x (query-row half h in 0..1).
Each core computes out[b, h*2048:(h+1)*2048, :].  The host feeds each
core x[b]^T with columns rotated so the core's own rows come first;
row order of K/V is softmax-invariant.
"""

import os
import sys

for _p in ("/opt/trn_rl_repo", "/opt/pypackages"):
    if _p not in sys.path and os.path.isdir(_p):
        sys.path.append(_p)

import numpy as np

import concourse.bass as bass
import concourse.tile as tile
from concourse import bacc, mybir

F32 = mybir.dt.float32
F32R = mybir.dt.float32r
F16 = mybir.dt.float16
BF16 = mybir.dt.bfloat16
EXP = mybir.ActivationFunctionType.Exp
LN = mybir.ActivationFunctionType.Ln
ADD = mybir.AluOpType.add
SUB = mybir.AluOpType.subtract
MULT = mybir.AluOpType.mult
IS_GT = mybir.AluOpType.is_gt

B, S, D, KD = 4, 4096, 256, 64
NCORES = 8
HALF = S // 2            # query rows per core (2048)
NJC = S // 128           # 32 j-chunks of 128 keys
BLK = 1024               # i-columns per row-block
NBLK = HALF // BLK       # 2 row-blocks per core
NIC = BLK // 128         # 8 output chunks of 128 rows per block
LAG = 2                  # accumulation matmuls trail producers by LAG chunks
M_SHIFT = 30.0           # keeps f32 den and bf16 e in range (|s| <= ~65)
LN_S = float(np.log(S))


def build_program(repeat: int = 1) -> bass.Bass:
    """repeat>1 builds the same kernel body N times back-to-back (timing aid).
    Consecutive bodies software-pipeline: B(h1) of body k runs inside P1 of
    body k+1."""
    nc = bacc.Bacc("TRN2", target_bir_lowering=False, debug=False)

    xT_d = nc.dram_tensor("xT16", [D, S], F16, kind="ExternalInput")
    wq_d = nc.dram_tensor("Wq16", [D, KD], F16, kind="ExternalInput")
    wk_d = nc.dram_tensor("Wk16", [D, KD], F16, kind="ExternalInput")
    wv_d = nc.dram_tensor("Wv16", [D, KD], F16, kind="ExternalInput")
    bqc_d = nc.dram_tensor("bq_col", [KD, 1], F32, kind="ExternalInput")
    bkc_d = nc.dram_tensor("bk_col", [KD, 1], F32, kind="ExternalInput")
    bvr_d = nc.dram_tensor("bv16_row", [1, KD], F16, kind="ExternalInput")
    ones_d = nc.dram_tensor("ones", [1, S], F16, kind="ExternalInput")
    out_d = nc.dram_tensor("out", [HALF, KD], F32, kind="ExternalOutput")

    ident_d = nc.inline_tensor(np.eye(128, dtype=np.float32), name="ident")

    with tile.TileContext(nc) as tc:
        with (
            tc.tile_pool(name="const", bufs=1) as cst,
            tc.tile_pool(name="vpool", bufs=1) as vpool,
            tc.tile_pool(name="epool", bufs=46) as epool,
            tc.tile_pool(name="work", bufs=4) as work,
            tc.tile_pool(name="up", bufs=2, space="PSUM") as up,
            tc.tile_pool(name="denp", bufs=1, space="PSUM") as denp,
            tc.tile_pool(name="op", bufs=1, space="PSUM") as op_,
        ):
            prev = None  # state of the previous repeat body awaiting B(h1)

            def emit_B_front(st, jc):
                """u' scores + mask-multiply for chunk jc of block h=1|0."""
                jsl = slice(jc * 128, (jc + 1) * 128)
                u = up.tile([128, BLK], F32, tag="u", name="u")
                for it in range(2):
                    osl = slice(it * 512, (it + 1) * 512)
                    nc.tensor.matmul(u[:, osl], st["kT"][:, jsl],
                                     st["qT"][:, osl], start=True, stop=True)
                g = work.tile([128, BLK], BF16, tag="g", name="g")
                nc.vector.scalar_tensor_tensor(g[:], u[:], 0.0,
                                               st["e"][jc][:], IS_GT, MULT)
                st["g"][jc] = g

            def emit_B_acc(st, jc):
                for it in range(2):
                    osl = slice(it * 512, (it + 1) * 512)
                    nc.tensor.matmul(st["o"][:, osl], st["v"][jc][:],
                                     st["g"][jc][:, osl],
                                     start=(jc == 0), stop=(jc == NJC - 1))
                st["g"][jc] = None

            def emit_epi(st):
                h, o_ps, dsb, ident = st["h"], st["o"], st["den_sb"], st["ident"]
                oT = cst.tile([KD, BLK], F32, tag=f"oT{h}", name="oT")
                nc.vector.tensor_copy(oT[:], o_ps[:])
                tp = up.tile([128, BLK], F32, tag="u", name="tp")
                for ic in range(NIC):
                    nc.tensor.transpose(tp[:, ic * 64:(ic + 1) * 64],
                                        oT[:, ic * 128:(ic + 1) * 128],
                                        ident[0:KD, 0:KD])
                for ic in range(NIC):
                    nc.tensor.transpose(tp[:, 512 + ic:513 + ic],
                                        dsb[0:1, ic * 128:(ic + 1) * 128],
                                        ident[0:1, 0:1])
                inv = cst.tile([128, NIC], F32, tag=f"inv{h}", name="inv")
                nc.vector.reciprocal(inv[:], tp[:, 512:512 + NIC])
                for ic in range(NIC):
                    o_sb = work.tile([128, KD], F32, tag="o_sb", name="o_sb")
                    nc.vector.tensor_scalar(o_sb[:],
                                            tp[:, ic * 64:(ic + 1) * 64],
                                            inv[:, ic:ic + 1], None, MULT)
                    r0 = h * BLK + ic * 128
                    nc.sync.dma_start(out_d[r0:r0 + 128, :], o_sb[:])

            for rep in range(repeat):
                par = rep % 2

                # ---- input DMAs (fp16 from host) ----------------------------
                x16 = []
                for dc in range(2):
                    t = cst.tile([128, S], F16, tag=f"x16_{dc}", name="x16")
                    nc.sync.dma_start(t[:, 0:HALF],
                                      xT_d[dc * 128:(dc + 1) * 128, 0:HALF])
                    nc.sync.dma_start(t[:, HALF:S],
                                      xT_d[dc * 128:(dc + 1) * 128, HALF:S])
                    x16.append(t)
                w16 = {}
                for dc in range(2):
                    for (m, dram) in (("wq", wq_d), ("wk", wk_d), ("wv", wv_d)):
                        t = cst.tile([128, KD], F16, tag=f"{m}16_{dc}", name="w16")
                        nc.sync.dma_start(t[:], dram[dc * 128:(dc + 1) * 128, :])
                        w16[(m, dc)] = t
                bq_c = cst.tile([KD, 1], F32, tag="bq_c")
                nc.sync.dma_start(bq_c[:], bqc_d[:])
                bk_c = cst.tile([KD, 1], F32, tag="bk_c")
                nc.sync.dma_start(bk_c[:], bkc_d[:])
                bv16_r = cst.tile([1, KD], F16, tag="bv16_r")
                nc.sync.dma_start(bv16_r[:], bvr_d[:])
                ident = cst.tile([128, 128], F32, tag="ident")
                nc.sync.dma_start(ident[:], ident_d.ap())
                ones16_row = cst.tile([1, 128], F16, tag="ones16_row")
                nc.vector.memset(ones16_row[:], 1.0)
                ones_colb = cst.tile([128, 1], BF16, tag="ones_colb")
                nc.vector.memset(ones_colb[:], 1.0)
                mshift_col = cst.tile([128, 1], F32, tag="mshift_col")
                nc.vector.memset(mshift_col[:], -M_SHIFT)

                kT = cst.tile([KD + 1, S], F16, tag=f"kT16_{par}", name="kT")
                nc.sync.dma_start(kT[KD:KD + 1, :], ones_d[:])
                qT = [cst.tile([KD + 1, BLK], F16, name=f"qT16_{h}",
                               tag=f"qT16_{h}_{par}") for h in range(NBLK)]
                for h in range(NBLK):
                    # zero aug row: pass A runs the same 65-row matmul as
                    # pass B (fp16 needs contraction >= 65 for full PE rate)
                    nc.vector.memset(qT[h][KD:KD + 1, :], 0.0)
                v16 = [vpool.tile([128, KD], BF16, name=f"v{jc}",
                                  tag=f"v{jc}_{par}") for jc in range(NJC)]

                # ---- P0: projections (PE-dense) -----------------------------
                for it in range(S // 512):
                    sl = slice(it * 512, (it + 1) * 512)
                    k_ps = up.tile([128, BLK], F32, tag="u", name="k_ps")
                    nc.tensor.matmul(k_ps[0:KD, 0:512], w16[("wk", 0)][:],
                                     x16[0][:, sl], start=True, stop=False)
                    nc.tensor.matmul(k_ps[0:KD, 0:512], w16[("wk", 1)][:],
                                     x16[1][:, sl], start=False, stop=True)
                    nc.vector.tensor_scalar(kT[0:KD, sl], k_ps[0:KD, 0:512],
                                            bk_c[:], None, ADD)
                for it in range(HALF // 512):
                    sl = slice(it * 512, (it + 1) * 512)
                    h, r = divmod(it * 512, BLK)
                    q_ps = up.tile([128, BLK], F32, tag="u", name="q_ps")
                    nc.tensor.matmul(q_ps[0:KD, 0:512], w16[("wq", 0)][:],
                                     x16[0][:, sl], start=True, stop=False)
                    nc.tensor.matmul(q_ps[0:KD, 0:512], w16[("wq", 1)][:],
                                     x16[1][:, sl], start=False, stop=True)
                    nc.vector.tensor_scalar(qT[h][0:KD, r:r + 512],
                                            q_ps[0:KD, 0:512], bq_c[:],
                                            None, ADD)
                for bb in range(2):
                    v_ps = up.tile([128, BLK], F32, tag="u", name="v_ps")
                    for m in range(16):
                        jc = bb * 16 + m
                        sl = slice(jc * 128, (jc + 1) * 128)
                        vsl = slice(m * 64, (m + 1) * 64)
                        nc.tensor.matmul(v_ps[:, vsl], x16[0][:, sl],
                                         w16[("wv", 0)][:],
                                         start=True, stop=False)
                        nc.tensor.matmul(v_ps[:, vsl], x16[1][:, sl],
                                         w16[("wv", 1)][:],
                                         start=False, stop=False)
                        nc.tensor.matmul(v_ps[:, vsl], ones16_row[:],
                                         bv16_r[:], start=False, stop=True)
                    for m in range(16):
                        nc.vector.tensor_copy(v16[bb * 16 + m][:],
                                              v_ps[:, m * 64:(m + 1) * 64])

                e_tiles = [[None] * NJC for _ in range(NBLK)]
                den_sb = [None] * NBLK

                def emit_A_front(h, jc, den_ps):
                    jsl = slice(jc * 128, (jc + 1) * 128)
                    u = up.tile([128, BLK], F32, tag="u", name="u")
                    for it in range(2):
                        osl = slice(it * 512, (it + 1) * 512)
                        nc.tensor.matmul(u[:, osl], kT[:, jsl],
                                         qT[h][:, osl],
                                         start=True, stop=True)
                    e = epool.tile([128, BLK], BF16, tag="e", name="e")
                    nc.scalar.activation(e[:], u[:], EXP, bias=mshift_col[:])
                    e_tiles[h][jc] = e

                def emit_A_acc(h, jc, den_ps):
                    for it in range(2):
                        osl = slice(it * 512, (it + 1) * 512)
                        nc.tensor.matmul(den_ps[0:1, osl], ones_colb[:],
                                         e_tiles[h][jc][:, osl],
                                         start=(jc == 0), stop=(jc == NJC - 1))

                def emit_t(h, den_ps):
                    dsb = cst.tile([1, BLK], F32, tag=f"den_sb{h}_{par}",
                                   name="dsb")
                    nc.vector.tensor_copy(dsb[:], den_ps[:])
                    den_sb[h] = dsb
                    lnd = cst.tile([1, BLK], F32, tag=f"lnd{h}", name="lnd")
                    nc.scalar.activation(lnd[:], den_ps[:], LN)
                    # aug row: -t = (lnS - M) - ln(den)
                    nc.vector.tensor_scalar(qT[h][KD:KD + 1, :], lnd[:],
                                            LN_S - M_SHIFT, -1.0, SUB, MULT)

                # ---- P1: A(h0) || B(h1 of previous body) --------------------
                den0 = denp.tile([1, BLK], F32, tag="den", name="den0")
                for jc in range(NJC):
                    emit_A_front(0, jc, den0)
                    if prev is not None:
                        emit_B_front(prev, jc)
                    if jc >= LAG:
                        emit_A_acc(0, jc - LAG, den0)
                        if prev is not None:
                            emit_B_acc(prev, jc - LAG)
                for jc in range(NJC - LAG, NJC):
                    emit_A_acc(0, jc, den0)
                    if prev is not None:
                        emit_B_acc(prev, jc)
                emit_t(0, den0)
                if prev is not None:
                    emit_epi(prev)
                    prev = None

                # ---- P2: A(h1) || B(h0) -------------------------------------
                den1 = denp.tile([1, BLK], F32, tag="den", name="den1")
                o0 = op_.tile([KD, BLK], F32, tag="o", name="o0")
                st0 = {"h": 0, "kT": kT, "qT": qT[0], "v": v16,
                       "e": e_tiles[0], "g": [None] * NJC, "o": o0,
                       "den_sb": None, "ident": ident}
                for jc in range(NJC):
                    emit_A_front(1, jc, den1)
                    emit_B_front(st0, jc)
                    if jc >= LAG:
                        emit_A_acc(1, jc - LAG, den1)
                        emit_B_acc(st0, jc - LAG)
                for jc in range(NJC - LAG, NJC):
                    emit_A_acc(1, jc, den1)
                    emit_B_acc(st0, jc)
                emit_t(1, den1)
                st0["den_sb"] = den_sb[0]
                emit_epi(st0)

                o1 = op_.tile([KD, BLK], F32, tag="o", name="o1")
                prev = {"h": 1, "kT": kT, "qT": qT[1], "v": v16,
                        "e": e_tiles[1], "g": [None] * NJC, "o": o1,
                        "den_sb": den_sb[1], "ident": ident}

            # ---- flush: B(h1) of the last body --------------------------
            for jc in range(NJC):
                emit_B_front(prev, jc)
                if jc >= LAG:
                    emit_B_acc(prev, jc - LAG)
            for jc in range(NJC - LAG, NJC):
                emit_B_acc(prev, jc)
            emit_epi(prev)

    nc.compile()
    return nc


# ---------------------------------------------------------------------------
# Host side: shard, run on 8 cores, gather.
# ---------------------------------------------------------------------------

_CACHE: dict = {}


def _in_maps(x, Wq, bq, Wk, bk, Wv, bv):
    maps = []
    wq16 = np.ascontiguousarray(np.asarray(Wq, np.float32).astype(np.float16))
    wk16 = np.ascontiguousarray(np.asarray(Wk, np.float32).astype(np.float16))
    wv16 = np.ascontiguousarray(np.asarray(Wv, np.float32).astype(np.float16))
    bqc = np.ascontiguousarray(np.asarray(bq, np.float32).reshape(KD, 1))
    bkc = np.ascontiguousarray(np.asarray(bk, np.float32).reshape(KD, 1))
    bv16 = np.ascontiguousarray(
        np.asarray(bv, np.float32).astype(np.float16).reshape(1, KD))
    for c in range(NCORES):
        b, h = divmod(c, 2)
        xb = np.asarray(x[b], dtype=np.float32)
        # rotate rows so this core's query rows come first, then transpose
        xrot = np.roll(xb, -h * HALF, axis=0)
        maps.append({
            "xT16": np.ascontiguousarray(xrot.T.astype(np.float16)),
            "Wq16": wq16,
            "Wk16": wk16,
            "Wv16": wv16,
            "bq_col": bqc,
            "bk_col": bkc,
            "bv16_row": bv16,
            "ones": np.ones((1, S), dtype=np.float16),
        })
    return maps


def get_runner():
    """Build the program once and return (nc, run_fn).

    run_fn(in_maps) -> list of per-core output dicts.  The jitted PJRT
    callable is cached so repeated kernel() calls don't recompile.
    """
    if "runner" in _CACHE:
        return _CACHE["runner"]

    nc = build_program()

    import jax
    from jax.sharding import Mesh, PartitionSpec
    from jax.experimental.shard_map import shard_map
    from concourse import bass2jax
    from concourse import mybir as _mybir

    bass2jax.install_neuronx_cc_hook()

    partition_name = nc.partition_id_tensor.name if nc.partition_id_tensor else None
    in_names, out_names, out_avals = [], [], []
    for alloc in nc.m.functions[0].allocations:
        if not isinstance(alloc, _mybir.MemoryLocationSet):
            continue
        name = alloc.memorylocations[0].name
        if alloc.kind == "ExternalInput":
            if name != partition_name:
                in_names.append(name)
        elif alloc.kind == "ExternalOutput":
            out_names.append(name)
            out_avals.append(jax.core.ShapedArray(
                tuple(alloc.tensor_shape), _mybir.dt.np(alloc.dtype)))
    n_params = len(in_names)
    all_names = in_names + out_names
    if partition_name is not None:
        all_names = all_names + [partition_name]

    def _body(*args):
        operands = list(args)
        if partition_name is not None:
            operands.append(bass2jax.partition_id_tensor())
        outs = bass2jax._bass_exec_p.bind(
            *operands,
            out_avals=tuple(out_avals),
            in_names=tuple(all_names),
            out_names=tuple(out_names),
            lowering_input_output_aliases=(),
            sim_require_finite=False,
            sim_require_nnan=False,
            nc=nc,
        )
        return tuple(outs)

    # Bust any HLO-module-level executable caching when the program changes:
    # the jit module name includes a content hash of the BIR.
    import hashlib
    _body.__name__ = "body_" + hashlib.sha256(nc.to_json_bytes()).hexdigest()[:12]

    devices = jax.devices()[:NCORES]
    mesh = Mesh(np.asarray(devices), ("core",))
    n_outs = len(out_names)
    sharded = jax.jit(shard_map(
        _body, mesh=mesh,
        in_specs=(PartitionSpec("core"),) * (n_params + n_outs),
        out_specs=(PartitionSpec("core"),) * n_outs,
        check_rep=False,
    ), keep_unused=True)

    def run_fn(maps):
        concat_in = [
            np.concatenate([np.asarray(maps[c][nm]) for c in range(NCORES)], axis=0)
            for nm in in_names
        ]
        concat_zero = [
            np.zeros((NCORES * av.shape[0], *av.shape[1:]), av.dtype)
            for av in out_avals
        ]
        outs = sharded(*concat_in, *concat_zero)
        return [
            {nm: np.asarray(outs[i]).reshape(NCORES, *out_avals[i].shape)[c]
             for i, nm in enumerate(out_names)}
            for c in range(NCORES)
        ]

    _CACHE["runner"] = (nc, run_fn, sharded, in_names, out_avals, out_names)
    return _CACHE["runner"]


def kernel(x, Wq, bq, Wk, bk, Wv, bv):
    _, run_fn, *_ = get_runner()
    results = run_fn(_in_maps(x, Wq, bq, Wk, bk, Wv, bv))
    out = np.empty((B, S, KD), dtype=np.float32)
    for c in range(NCORES):
        b, h = divmod(c, 2)
        out[b, h * HALF:(h + 1) * HALF, :] = results[c]["out"]
    return out



# revision 20
# speedup vs baseline: 12.6409x; 12.6409x over previous
"""Trainium2 Bass kernel for nn_DifferentialAttention (sparse attention).

Reference computation (per batch element b):
    Q = x @ Wq + bq ; K = x @ Wk + bk ; V = x @ Wv + bv        [S, KD]
    scores  = Q @ K^T                                          [S, S]
    weights = softmax(scores, axis=-1)
    mask    = weights > mean(weights, axis=-1, keepdims=True)
    out     = (weights * mask) @ V                             [S, KD]

Single-pass formulation ([j,i] layout, j = key, i = query):
  * mean(softmax row) == 1/S exactly, so mask_ij = [w_ij > 1/S]
      = [e_ij > den_i / S],  e_ij = exp(s_ij - M), den_i = sum_j e_ij.
  * out_i = (sum_j e_ij * mask_ij * V_j) / den_i   -- M cancels.

v4 design (per row-block h of 1024 query columns, 16 chunk-pairs x 2 osl):
  A:   u[:,0:512]   = kT2[0:64,  c]^T q (row-tile T0)   \  concurrent K=64
       u[:,512:1024]= kT2[64:128,c]^T q (row-tile T8)   /  pair: ~2x PE rate
       e = exp(u - M) -> bf16 [128,1024] (Act)
       merged = e_lo + e_hi                (DVE TT bf16, 2x mode)
       den[0:64,osl] += ones64^T merged    (PE, M=64 so it shares the
                        128x64 PE tiling mode with the O matmuls)
  c_b: after den(h): c_ps = invS_row^T den_sb (rank-1 K=1 matmuls),
       c_b[h][osl] = bf16 copy  -- the threshold den_i/S broadcast to
       128 partitions (DVE cannot partition-broadcast; PE can).
  G:   m = (e > c_b)  (DVE TT is_gt bf16, 2x)    -- replaces the old
       g = m * e      (DVE TT mult bf16, 2x)        PE score recompute
  O:   o[0:64,osl] += v16[2c]^T g_lo ; += v16[2c+1]^T g_hi  (PE)
  epilogue: transpose o via PE, scale rows by 1/den, DMA out.

  Engine balance per body (HW NTFF, PE mostly at 1.2 GHz):  PE ~114us,
  DVE ~134us (threshold TTs 83 + merge 28 + copies), Act ~80us (exp).
  Measured 143us/body on-device vs 292us for the two-pass baseline.
  Rejected variants (measured slower): den matmuls reading raw e halves
  (global +200ns on every DVE op from SBUF contention), den sharing the
  o PSUM tile with skip_group_check (phase-boundary serialization).

Key measured facts driving this design (HW microbench + NTFF traces):
  * PE is throttled to 1.2 GHz (K=4/8) ~94% of the time under this
    workload; PE cycles are the currency.  Old kernel: ~290k cyc/body
    (A 65k + den 65k + B-recompute 65k + O 65k + proj/epi).
  * DVE tensor_tensor DOES get 2x mode for bf16/fp16 (603ns/[128,1024]);
    scalar_tensor_tensor does not (1146ns).  So threshold-on-DVE costs
    ~the same as the old single stt while freeing 65k PE cyc.
  * K=64 row-tiled MM pairs run ~2x (147ns/MM warm vs 218 serial).
  * fp16 e overflows (row maxes spread 19..65) -- e stays bf16, M=30.

Sharding: 8 cores = (batch b in 0..3) x (query-row half h in 0..1).
Each core computes out[b, h*2048:(h+1)*2048, :].  The host feeds each
core x[b]^T with columns rotated so the core's own rows come first;
row order of K/V is softmax-invariant.
"""

import os
import sys

for _p in ("/opt/trn_rl_repo", "/opt/pypackages"):
    if _p not in sys.path and os.path.isdir(_p):
        sys.path.append(_p)

import numpy as np

import concourse.bass as bass
import concourse.tile as tile
from concourse import bacc, mybir

F32 = mybir.dt.float32
F16 = mybir.dt.float16
BF16 = mybir.dt.bfloat16
EXP = mybir.ActivationFunctionType.Exp
IDENT = mybir.ActivationFunctionType.Identity
ADD = mybir.AluOpType.add
MULT = mybir.AluOpType.mult
IS_GT = mybir.AluOpType.is_gt

B, S, D, KD = 4, 4096, 256, 64
NCORES = 8
HALF = S // 2            # query rows per core (2048)
NJC = S // 128           # 32 j-chunks of 128 keys
NPAIR = NJC // 2         # 16 chunk pairs
BLK = 1024               # i-columns per row-block
NBLK = HALF // BLK       # 2 row-blocks per core
NIC = BLK // 128         # 8 output chunks of 128 rows per block
NIT = NPAIR * 2          # 32 (c, osl) iterations per row-block
LAG = 3                  # den/O matmuls trail their producers
NEBUF = 40               # rotating e-tile buffers (40 x 256KB = 10MB)
M_SHIFT = 30.0


def build_program(repeat: int = 1) -> bass.Bass:
    """repeat>1 builds the same kernel body N times back-to-back (timing aid).
    Consecutive bodies software-pipeline: G/O(h1) of body k runs inside P1 of
    body k+1."""
    nc = bacc.Bacc("TRN2", target_bir_lowering=False, debug=False)

    xT_d = nc.dram_tensor("xT16", [D, S], F16, kind="ExternalInput")
    wq_d = nc.dram_tensor("Wq16", [D, KD], F16, kind="ExternalInput")
    wk_d = nc.dram_tensor("Wk16", [D, KD], F16, kind="ExternalInput")
    wv_d = nc.dram_tensor("Wv16", [D, KD], F16, kind="ExternalInput")
    bqc_d = nc.dram_tensor("bq_col", [KD, 1], F32, kind="ExternalInput")
    bk2_d = nc.dram_tensor("bk_col2", [128, 1], F32, kind="ExternalInput")
    bvb_d = nc.dram_tensor("bv_bc2", [128, 2 * KD], BF16, kind="ExternalInput")
    out_d = nc.dram_tensor("out", [HALF, KD], F32, kind="ExternalOutput")

    ident_d = nc.inline_tensor(np.eye(128, dtype=np.float32), name="ident")

    with tile.TileContext(nc) as tc:
        with (
            tc.tile_pool(name="const", bufs=1) as cst,
            tc.tile_pool(name="vpool", bufs=1) as vpool,
            tc.tile_pool(name="epool", bufs=NEBUF) as epool,
            tc.tile_pool(name="mpool", bufs=4) as mpool,
            tc.tile_pool(name="gpool", bufs=6) as gpool,
            tc.tile_pool(name="mgpool", bufs=5) as mgpool,
            tc.tile_pool(name="work", bufs=4) as work,
            tc.tile_pool(name="up", bufs=2, space="PSUM") as up,
            tc.tile_pool(name="odp", bufs=1, space="PSUM") as odp,
            tc.tile_pool(name="denp", bufs=1, space="PSUM") as denp,
        ):
            prev = None  # state of the previous repeat body awaiting G/O(h1)

            def emit_G(st, k):
                """threshold chunk-pair k of block h: m=(e>c), g=m*e."""
                osl = k % 2
                e = st["e"][k]
                m = mpool.tile([128, BLK], BF16, tag="m", name="m")
                nc.vector.tensor_tensor(m[:], e[:], st["cb"][osl][:], IS_GT)
                g = gpool.tile([128, BLK], BF16, tag="g", name="g")
                nc.vector.tensor_tensor(g[:], m[:], e[:], MULT)
                st["g"][k] = g
                st["e"][k] = None

            def emit_O(st, k):
                c, osl = divmod(k, 2)
                osl_sl = slice(osl * 512, (osl + 1) * 512)
                g = st["g"][k]
                nc.tensor.matmul(st["o"][0:KD, osl_sl], st["v"][2 * c][:],
                                 g[:, 0:512], start=(c == 0), stop=False)
                nc.tensor.matmul(st["o"][0:KD, osl_sl], st["v"][2 * c + 1][:],
                                 g[:, 512:1024], start=False,
                                 stop=(c == NPAIR - 1))
                st["g"][k] = None

            def emit_epi(st):
                h, od_ps, dsb, ident = st["h"], st["o"], st["den_sb"], st["ident"]
                oT = cst.tile([KD, BLK], F32, tag=f"oT{h}", name="oT")
                nc.vector.tensor_copy(oT[:], od_ps[0:KD, :])
                tp = up.tile([128, BLK], F32, tag="u", name="tp")
                for ic in range(NIC):
                    nc.tensor.transpose(tp[:, ic * 64:(ic + 1) * 64],
                                        oT[:, ic * 128:(ic + 1) * 128],
                                        ident[0:KD, 0:KD])
                for ic in range(NIC):
                    nc.tensor.transpose(tp[:, 512 + ic:513 + ic],
                                        dsb[0:1, ic * 128:(ic + 1) * 128],
                                        ident[0:1, 0:1])
                inv = cst.tile([128, NIC], F32, tag=f"inv{h}", name="inv")
                nc.vector.reciprocal(inv[:], tp[:, 512:512 + NIC])
                for ic in range(NIC):
                    o_sb = work.tile([128, KD], F32, tag="o_sb", name="o_sb")
                    nc.vector.tensor_scalar(o_sb[:],
                                            tp[:, ic * 64:(ic + 1) * 64],
                                            inv[:, ic:ic + 1], None, MULT)
                    r0 = h * BLK + ic * 128
                    nc.sync.dma_start(out_d[r0:r0 + 128, :], o_sb[:])

            # ---- constants: DMA'd / memset once, reused by every body ----
            w16 = {}
            for dc in range(2):
                for (m, dram) in (("wq", wq_d), ("wk", wk_d), ("wv", wv_d)):
                    t = cst.tile([128, KD], F16, tag=f"{m}16_{dc}", name="w16")
                    nc.sync.dma_start(t[:], dram[dc * 128:(dc + 1) * 128, :])
                    w16[(m, dc)] = t
            bq_c = cst.tile([KD, 1], F32, tag="bq_c")
            nc.sync.dma_start(bq_c[:], bqc_d[:])
            bk_c2 = cst.tile([128, 1], F32, tag="bk_c2")
            nc.sync.dma_start(bk_c2[:], bk2_d[:])
            bv_bc2 = cst.tile([128, 2 * KD], BF16, tag="bv_bc2")
            nc.sync.dma_start(bv_bc2[:], bvb_d[:])
            ident = cst.tile([128, 128], F32, tag="ident")
            nc.sync.dma_start(ident[:], ident_d.ap())
            ones64 = cst.tile([128, KD], BF16, tag="ones64")
            nc.vector.memset(ones64[:], 1.0)
            invS_row = cst.tile([1, 128], F32, tag="invS_row")
            nc.vector.memset(invS_row[:], 1.0 / S)
            mshift_col = cst.tile([128, 1], F32, tag="mshift_col")
            nc.vector.memset(mshift_col[:], -M_SHIFT)

            for rep in range(repeat):
                par = rep % 2

                # ---- input DMAs (fp16 from host) ----------------------------
                x16 = []
                for dc in range(2):
                    t = cst.tile([128, S], F16, tag=f"x16_{dc}", name="x16")
                    nc.sync.dma_start(t[:, 0:HALF],
                                      xT_d[dc * 128:(dc + 1) * 128, 0:HALF])
                    nc.sync.dma_start(t[:, HALF:S],
                                      xT_d[dc * 128:(dc + 1) * 128, HALF:S])
                    x16.append(t)

                # kT2: rows 0-63 = K^T even chunks, 64-127 = odd chunks
                kT2 = cst.tile([128, NPAIR * 128], F16, tag=f"kT2_{par}",
                               name="kT2")
                qT2 = [cst.tile([128, BLK], F16, name=f"qT2_{h}",
                                tag=f"qT2_{h}_{par}") for h in range(NBLK)]
                vp16 = [vpool.tile([128, 128], BF16, name=f"vp{c}",
                                   tag=f"vp{c}_{par}") for c in range(NPAIR)]
                v16 = [vp16[jc // 2][:, (jc % 2) * KD:(jc % 2 + 1) * KD]
                       for jc in range(NJC)]

                # ---- P0: projections ---------------------------------------
                # K: even chunks -> psum rows 0-63, odd -> rows 64-127 (T1)
                for it in range(S // 512):
                    sl0 = it * 512
                    k_ps = up.tile([128, BLK], F32, tag="u", name="k_ps")
                    for par_eo in range(2):
                        # chunks 4it + {0,2} (par_eo=0) or 4it + {1,3}
                        rb = par_eo * 64
                        dst = k_ps[rb:rb + 64, 0:256].rearrange(
                            "p (b c) -> p b c", c=128)
                        for dc in range(2):
                            rhs = x16[dc][:, sl0:sl0 + 512].rearrange(
                                "p (b two c) -> p two b c", two=2,
                                c=128)[:, par_eo, :, :]
                            nc.tensor.matmul(dst, w16[("wk", dc)][:], rhs,
                                             start=(dc == 0), stop=(dc == 1))
                    nc.scalar.activation(kT2[:, it * 256:(it + 1) * 256],
                                         k_ps[:, 0:256], IDENT,
                                         bias=bk_c2[:])
                # Q: rows 0-63, then DMA-duplicate to rows 64-127
                for it in range(HALF // 512):
                    sl = slice(it * 512, (it + 1) * 512)
                    h, r = divmod(it * 512, BLK)
                    q_ps = up.tile([128, BLK], F32, tag="u", name="q_ps")
                    nc.tensor.matmul(q_ps[0:KD, 0:512], w16[("wq", 0)][:],
                                     x16[0][:, sl], start=True, stop=False)
                    nc.tensor.matmul(q_ps[0:KD, 0:512], w16[("wq", 1)][:],
                                     x16[1][:, sl], start=False, stop=True)
                    nc.scalar.activation(qT2[h][0:KD, r:r + 512],
                                         q_ps[0:KD, 0:512], IDENT,
                                         bias=bq_c[:])
                for h in range(NBLK):
                    nc.sync.dma_start(qT2[h][64:128, :], qT2[h][0:64, :])
                # V: [j, KD] per chunk; bias via TT add of broadcast row
                for bb in range(2):
                    v_ps = up.tile([128, BLK], F32, tag="u", name="v_ps")
                    for m in range(16):
                        jc = bb * 16 + m
                        sl = slice(jc * 128, (jc + 1) * 128)
                        vsl = slice(m * 64, (m + 1) * 64)
                        nc.tensor.matmul(v_ps[:, vsl], x16[0][:, sl],
                                         w16[("wv", 0)][:],
                                         start=True, stop=False)
                        nc.tensor.matmul(v_ps[:, vsl], x16[1][:, sl],
                                         w16[("wv", 1)][:],
                                         start=False, stop=True)
                    for m in range(8):
                        c = bb * 8 + m
                        nc.vector.tensor_tensor(
                            vp16[c][:],
                            v_ps[:, m * 128:(m + 1) * 128],
                            bv_bc2[:], ADD)

                e_tiles = [[None] * NIT for _ in range(NBLK)]
                mg_tiles = [[None] * NIT for _ in range(NBLK)]
                den_sb = [None] * NBLK
                cb_tiles = [[None, None] for _ in range(NBLK)]

                def emit_A(h, k):
                    c, osl = divmod(k, 2)
                    jsl = slice(c * 128, (c + 1) * 128)
                    osl_sl = slice(osl * 512, (osl + 1) * 512)
                    u = up.tile([128, BLK], F32, tag="u", name="u")
                    nc.tensor.matmul(u[:, 0:512], kT2[0:64, jsl],
                                     qT2[h][0:64, osl_sl],
                                     start=True, stop=True)
                    nc.tensor.matmul(u[:, 512:1024], kT2[64:128, jsl],
                                     qT2[h][64:128, osl_sl],
                                     start=True, stop=True)
                    e = epool.tile([128, BLK], BF16, tag="e", name="e")
                    nc.scalar.activation(e[:], u[:], EXP, bias=mshift_col[:])
                    e_tiles[h][k] = e
                    mg = mgpool.tile([128, 512], BF16, tag="mg", name="mg")
                    nc.vector.tensor_tensor(mg[:], e[:, 0:512],
                                            e[:, 512:1024], ADD)
                    mg_tiles[h][k] = mg

                def emit_den(h, k, den_ps):
                    c, osl = divmod(k, 2)
                    osl_sl = slice(osl * 512, (osl + 1) * 512)
                    nc.tensor.matmul(den_ps[0:KD, osl_sl], ones64[:],
                                     mg_tiles[h][k][:],
                                     start=(c == 0), stop=(c == NPAIR - 1))
                    mg_tiles[h][k] = None

                def emit_cb(h, den_ps):
                    dsb = cst.tile([1, BLK], F32, tag=f"den_sb{h}_{par}",
                                   name="dsb")
                    nc.vector.tensor_copy(dsb[:], den_ps[0:1, :])
                    den_sb[h] = dsb
                    c_ps = up.tile([128, BLK], F32, tag="u", name="c_ps")
                    for osl in range(2):
                        osl_sl = slice(osl * 512, (osl + 1) * 512)
                        nc.tensor.matmul(c_ps[:, 0:512], invS_row[:],
                                         dsb[0:1, osl_sl],
                                         start=True, stop=True)
                        nc.tensor.matmul(c_ps[:, 512:1024], invS_row[:],
                                         dsb[0:1, osl_sl],
                                         start=True, stop=True)
                        cb = cst.tile([128, BLK], BF16, tag=f"cb{h}{osl}",
                                      name="cb")
                        nc.vector.tensor_copy(cb[:], c_ps[:])
                        cb_tiles[h][osl] = cb

                # ---- P1: A(h0) + den(h0) || G/O(h1 of previous body) --------
                # One PSUM tile per phase: rows 64-127 = den(A-half),
                # rows 0-63 = o(G-half).  Same PE mode, no bank overlap.
                den_p1 = denp.tile([KD, BLK], F32, tag="den", name="den_p1")
                od_p1 = odp.tile([KD, BLK], F32, tag="od", name="od_p1")
                if prev is not None:
                    prev["o"] = od_p1
                for k in range(NIT):
                    emit_A(0, k)
                    if prev is not None:
                        emit_G(prev, k)
                    if k >= LAG:
                        emit_den(0, k - LAG, den_p1)
                        if prev is not None:
                            emit_O(prev, k - LAG)
                for k in range(NIT - LAG, NIT):
                    emit_den(0, k, den_p1)
                    if prev is not None:
                        emit_O(prev, k)
                emit_cb(0, den_p1)
                if prev is not None:
                    emit_epi(prev)
                    prev = None

                # ---- P2: A(h1) + den(h1) || G/O(h0) -------------------------
                den_p2 = denp.tile([KD, BLK], F32, tag="den", name="den_p2")
                od_p2 = odp.tile([KD, BLK], F32, tag="od", name="od_p2")
                st0 = {"h": 0, "v": v16, "e": e_tiles[0], "g": [None] * NIT,
                       "o": od_p2, "cb": cb_tiles[0], "den_sb": den_sb[0],
                       "ident": ident}
                for k in range(NIT):
                    emit_A(1, k)
                    emit_G(st0, k)
                    if k >= LAG:
                        emit_den(1, k - LAG, den_p2)
                        emit_O(st0, k - LAG)
                for k in range(NIT - LAG, NIT):
                    emit_den(1, k, den_p2)
                    emit_O(st0, k)
                emit_cb(1, den_p2)
                emit_epi(st0)

                prev = {"h": 1, "v": v16, "e": e_tiles[1], "g": [None] * NIT,
                        "o": None, "cb": cb_tiles[1], "den_sb": den_sb[1],
                        "ident": ident}

            # ---- flush: G/O(h1) of the last body ------------------------
            od_f = odp.tile([KD, BLK], F32, tag="od", name="od_f")
            prev["o"] = od_f
            for k in range(NIT):
                emit_G(prev, k)
                if k >= LAG:
                    emit_O(prev, k - LAG)
            for k in range(NIT - LAG, NIT):
                emit_O(prev, k)
            emit_epi(prev)

    nc.compile()
    return nc


# ---------------------------------------------------------------------------
# Host side: shard, run on 8 cores, gather.
# ---------------------------------------------------------------------------

_CACHE: dict = {}


def _in_maps(x, Wq, bq, Wk, bk, Wv, bv):
    maps = []
    bf16 = mybir.dt.np(BF16)
    wq16 = np.ascontiguousarray(np.asarray(Wq, np.float32).astype(np.float16))
    wk16 = np.ascontiguousarray(np.asarray(Wk, np.float32).astype(np.float16))
    wv16 = np.ascontiguousarray(np.asarray(Wv, np.float32).astype(np.float16))
    bqc = np.ascontiguousarray(np.asarray(bq, np.float32).reshape(KD, 1))
    bkc2 = np.ascontiguousarray(
        np.tile(np.asarray(bk, np.float32).reshape(KD, 1), (2, 1)))
    bvb = np.ascontiguousarray(
        np.broadcast_to(np.tile(np.asarray(bv, np.float32), 2).reshape(1, 2 * KD),
                        (128, 2 * KD)).astype(bf16))
    for c in range(NCORES):
        b, h = divmod(c, 2)
        xb = np.asarray(x[b], dtype=np.float32)
        # rotate rows so this core's query rows come first, then transpose
        xrot = np.roll(xb, -h * HALF, axis=0)
        maps.append({
            "xT16": np.ascontiguousarray(xrot.T.astype(np.float16)),
            "Wq16": wq16,
            "Wk16": wk16,
            "Wv16": wv16,
            "bq_col": bqc,
            "bk_col2": bkc2,
            "bv_bc2": bvb,
        })
    return maps


def get_runner():
    """Build the program once and return (nc, run_fn).

    run_fn(in_maps) -> list of per-core output dicts.  The jitted PJRT
    callable is cached so repeated kernel() calls don't recompile.
    """
    if "runner" in _CACHE:
        return _CACHE["runner"]

    nc = build_program()

    import jax
    from jax.sharding import Mesh, PartitionSpec
    from jax.experimental.shard_map import shard_map
    from concourse import bass2jax
    from concourse import mybir as _mybir

    bass2jax.install_neuronx_cc_hook()

    partition_name = nc.partition_id_tensor.name if nc.partition_id_tensor else None
    in_names, out_names, out_avals = [], [], []
    for alloc in nc.m.functions[0].allocations:
        if not isinstance(alloc, _mybir.MemoryLocationSet):
            continue
        name = alloc.memorylocations[0].name
        if alloc.kind == "ExternalInput":
            if name != partition_name:
                in_names.append(name)
        elif alloc.kind == "ExternalOutput":
            out_names.append(name)
            out_avals.append(jax.core.ShapedArray(
                tuple(alloc.tensor_shape), _mybir.dt.np(alloc.dtype)))
    n_params = len(in_names)
    all_names = in_names + out_names
    if partition_name is not None:
        all_names = all_names + [partition_name]

    def _body(*args):
        operands = list(args)
        if partition_name is not None:
            operands.append(bass2jax.partition_id_tensor())
        outs = bass2jax._bass_exec_p.bind(
            *operands,
            out_avals=tuple(out_avals),
            in_names=tuple(all_names),
            out_names=tuple(out_names),
            lowering_input_output_aliases=(),
            sim_require_finite=False,
            sim_require_nnan=False,
            nc=nc,
        )
        return tuple(outs)

    # Bust any HLO-module-level executable caching when the program changes:
    # the jit module name includes a content hash of the BIR.
    import hashlib
    _body.__name__ = "body_" + hashlib.sha256(nc.to_json_bytes()).hexdigest()[:12]

    devices = jax.devices()[:NCORES]
    mesh = Mesh(np.asarray(devices), ("core",))
    n_outs = len(out_names)
    sharded = jax.jit(shard_map(
        _body, mesh=mesh,
        in_specs=(PartitionSpec("core"),) * (n_params + n_outs),
        out_specs=(PartitionSpec("core"),) * n_outs,
        check_rep=False,
    ), keep_unused=True)

    def run_fn(maps):
        concat_in = [
            np.concatenate([np.asarray(maps[c][nm]) for c in range(NCORES)], axis=0)
            for nm in in_names
        ]
        concat_zero = [
            np.zeros((NCORES * av.shape[0], *av.shape[1:]), av.dtype)
            for av in out_avals
        ]
        outs = sharded(*concat_in, *concat_zero)
        return [
            {nm: np.asarray(outs[i]).reshape(NCORES, *out_avals[i].shape)[c]
             for i, nm in enumerate(out_names)}
            for c in range(NCORES)
        ]

    _CACHE["runner"] = (nc, run_fn, sharded, in_names, out_avals, out_names)
    return _CACHE["runner"]


def kernel(x, Wq, bq, Wk, bk, Wv, bv):
    _, run_fn, *_ = get_runner()
    results = run_fn(_in_maps(x, Wq, bq, Wk, bk, Wv, bv))
    out = np.empty((B, S, KD), dtype=np.float32)
    for c in range(NCORES):
        b, h = divmod(c, 2)
        out[b, h * HALF:(h + 1) * HALF, :] = results[c]["out"]
    return out
